# revision 20
# baseline (speedup 1.0000x reference)
"""Distributed causal self-attention kernel for 8 TRN2 NeuronCores.

Sharding: core c handles batch b=c//2 and head-half hf=c%2 (8 of 16 heads).
Per core: qkv projection (bf16 matmuls into f32 PSUM), RoPE in f32r (PE
swap-matmul + DVE), causal flash attention in [tk, tq] layout (scores f32r;
exp -> bf16 weights; softmax sums via a ones-matmul; normalization deferred
to the y eviction), output projection in f32r producing the TRANSPOSED
partial output [C, T], then 4 chunked 2-way ReduceScatters within each batch
pair overlapping the projection. y stays resident in SBUF (no HBM round
trip). Multi-wait instructions are legalized into single-wait NoOps because
this container's walrus rejects them.
"""

import sys

sys.path.insert(0, "/opt/trn_rl_repo")

import ml_dtypes
import numpy as np

import concourse.bass as bass
import concourse.mybir as mybir
from concourse.bass_utils import run_bass_kernel_spmd
from concourse.tile import TileContext

# Problem constants (hardcoded; kernel.py must be self-contained)
B, T, C = 4, 2048, 2048
H, D = 16, 128
HL = 8  # local heads per core
CLOC = HL * D  # 1024 local y features
ROPE_BASE = 500000
P = 128
NCT = C // P  # 16 contraction tiles
TQC = 512  # tq chunk
NTQ = T // TQC  # 4
NTK = T // P  # 16 tk tiles
SCALE = 1.0 / float(np.sqrt(D))

f32 = mybir.dt.float32
f32r = mybir.dt.float32r
bf16 = mybir.dt.bfloat16
EXP = mybir.ActivationFunctionType.Exp


def build_nc(with_collective=True):
    nc = bass.Bass(target_bir_lowering=False, num_devices=8)

    # per-core parameters (host pre-arranged layouts, all contiguous DMAs)
    xT = nc.declare_dram_parameter("xT", [P, NCT, T], bf16, isOutput=False)
    wq = nc.declare_dram_parameter("wq", [HL, P, NCT, P], bf16, isOutput=False)
    wk = nc.declare_dram_parameter("wk", [HL, P, NCT, P], bf16, isOutput=False)
    wv = nc.declare_dram_parameter("wv", [HL, P, NCT, P], bf16, isOutput=False)
    wp = nc.declare_dram_parameter("wp", [P, HL, C], f32r, isOutput=False)
    cost = nc.declare_dram_parameter("cost", [P, T], f32r, isOutput=False)
    sint = nc.declare_dram_parameter("sint", [P, T], f32r, isOutput=False)
    mskt = nc.declare_dram_parameter("mskt", [P, 896], bf16, isOutput=False)
    swpm = nc.declare_dram_parameter("swpm", [P, P], f32r, isOutput=False)
    idnm = nc.declare_dram_parameter("idnm", [P, P], bf16, isOutput=False)
    onem = nc.declare_dram_parameter("onem", [P, P], f32r, isOutput=False)
    onbm = nc.declare_dram_parameter("onbm", [P, P], bf16, isOutput=False)
    outp = nc.declare_dram_parameter("out", [CLOC, T], f32, isOutput=True)

    poutT = nc.dram_tensor("poutT", [C, T], f32)
    rs_out = nc.dram_tensor("rs_out", [CLOC, T], f32)

    with TileContext(nc) as tc:
        with (
            tc.tile_pool(name="const", bufs=1) as cp,
            tc.tile_pool(name="ybuf", bufs=1) as yp,
        ):
            cos_sb = cp.tile([P, T], f32r, tag="cos")
            sin_sb = cp.tile([P, T], f32r, tag="sin")
            msk_sb = cp.tile([P, 896], bf16, tag="msk")
            swp_sb = cp.tile([P, P], f32r, tag="swp")
            idn_sb = cp.tile([P, P], bf16, tag="idn")
            ones_sb = cp.tile([P, P], f32r, tag="ones1")
            oneb_sb = cp.tile([P, P], bf16, tag="ones2")
            # y (normalized, f32r) stays resident across attention + proj
            y2_sb = yp.tile([P, HL * T], f32r, tag="y2")

            # ---------------- attention phase ----------------
            with (
                tc.tile_pool(name="xt", bufs=1) as xtp,
                tc.tile_pool(name="wqk", bufs=4) as wqkp,
                tc.tile_pool(name="qk", bufs=3) as qkp,
                tc.tile_pool(name="rtmp", bufs=2) as rtp,
                tc.tile_pool(name="e", bufs=3) as epl,
                tc.tile_pool(name="sum", bufs=2) as smp,
                tc.tile_pool(name="inv", bufs=2) as ivp,
                tc.tile_pool(name="ps", bufs=2, space="PSUM") as psp,
                tc.tile_pool(name="psa", bufs=2, space="PSUM") as psap,
                tc.tile_pool(name="psy", bufs=1, space="PSUM") as psyp,
                tc.tile_pool(name="pss", bufs=1, space="PSUM") as pssp,
            ):
                # prefetch head-0's wv before the big xt load so the PE
                # can start as soon as the first xt chunks land
                wv_first = wqkp.tile([P, NCT * P], bf16, tag="wqk")
                nc.sync.dma_start(
                    out=wv_first[:], in_=wv[0].rearrange("p n d -> p (n d)")
                )
                xt_sb = xtp.tile([P, NCT * T], bf16, tag="xt")
                for ci in range(0, NCT, 2):
                    nc.sync.dma_start(
                        out=xt_sb[:, ci * T : (ci + 2) * T],
                        in_=xT[:, ci : ci + 2, :].rearrange("p n t -> p (n t)"),
                    )
                # consts are needed later than xt/weights: issue after
                nc.sync.dma_start(out=idn_sb[:], in_=idnm[:, :])
                nc.sync.dma_start(out=cos_sb[:], in_=cost[:, :])
                nc.sync.dma_start(out=sin_sb[:], in_=sint[:, :])
                nc.sync.dma_start(out=msk_sb[:], in_=mskt[:, :])
                nc.sync.dma_start(out=swp_sb[:], in_=swpm[:, :])
                nc.sync.dma_start(out=ones_sb[:], in_=onem[:, :])
                nc.sync.dma_start(out=oneb_sb[:], in_=onbm[:, :])

                def qk_acc(w_sb, dst):
                    # dst[:, t] = sum_c w[c, d].T @ xT[c, t], two 2-bank halves
                    for hh in range(2):
                        ps = psap.tile([P, 2 * TQC], f32, tag="qkacc")
                        for ci in range(NCT):
                            lhsT = w_sb[:, ci * P : (ci + 1) * P]
                            for tt in range(2):
                                t0 = (hh * 2 + tt) * TQC
                                nc.tensor.matmul(
                                    ps[:, tt * TQC : (tt + 1) * TQC],
                                    lhsT,
                                    xt_sb[:, ci * T + t0 : ci * T + t0 + TQC],
                                    start=(ci == 0),
                                    stop=(ci == NCT - 1),
                                )
                        nc.vector.tensor_copy(
                            dst[:, hh * 2 * TQC : (hh + 1) * 2 * TQC], ps[:]
                        )

                def rope(src):
                    # in-place: src = src*cos + swap(src)*sin (sin table signed;
                    # the 1/sqrt(D) scale is folded into the exp activation)
                    for cc in range(NTQ):
                        sl = slice(cc * TQC, (cc + 1) * TQC)
                        ps = psp.tile([P, TQC], f32, tag="sc")
                        nc.tensor.matmul(
                            ps[:], swp_sb[:], src[:, sl], start=True, stop=True
                        )
                        tmp = rtp.tile([P, TQC], f32r, tag="rtmp")
                        nc.vector.tensor_mul(tmp[:], ps[:], sin_sb[:, sl])
                        nc.vector.tensor_mul(src[:, sl], src[:, sl], cos_sb[:, sl])
                        nc.vector.tensor_add(src[:, sl], src[:, sl], tmp[:])

                for hl in range(HL):
                    # v via the same machinery (transposed layout), then
                    # PE-transpose per 128-tile into natural [tk, d], all bf16
                    if hl == 0:
                        wv_sb = wv_first
                    else:
                        wv_sb = wqkp.tile([P, NCT * P], bf16, tag="wqk")
                        nc.sync.dma_start(
                            out=wv_sb[:], in_=wv[hl].rearrange("p n d -> p (n d)")
                        )
                    vT = qkp.tile([P, T], bf16, tag="qk")
                    qk_acc(wv_sb, vT)
                    v_sb = qkp.tile([P, T], bf16, tag="qk")
                    for j in range(NTK):
                        pst = psp.tile([P, P], bf16, tag="sc")
                        nc.tensor.transpose(
                            pst[:], vT[:, j * P : (j + 1) * P], idn_sb[:]
                        )
                        nc.vector.tensor_copy(v_sb[:, j * P : (j + 1) * P], pst[:])

                    wk_sb = wqkp.tile([P, NCT * P], bf16, tag="wqk")
                    nc.sync.dma_start(
                        out=wk_sb[:], in_=wk[hl].rearrange("p n d -> p (n d)")
                    )
                    rk = qkp.tile([P, T], f32r, tag="qk")
                    qk_acc(wk_sb, rk)
                    rope(rk)

                    wq_sb = wqkp.tile([P, NCT * P], bf16, tag="wqk")
                    nc.sync.dma_start(
                        out=wq_sb[:], in_=wq[hl].rearrange("p n d -> p (n d)")
                    )
                    rq = qkp.tile([P, T], f32r, tag="qk")
                    qk_acc(wq_sb, rq)
                    rope(rq)

                    for cc in range(NTQ):
                        qsl = slice(cc * TQC, (cc + 1) * TQC)
                        njt = 4 * cc + 4  # causal tk tiles
                        psy = psyp.tile([P, TQC], f32, tag="yac")
                        pss = pssp.tile([1, TQC], f32, tag="sums")
                        for j in range(njt):
                            ps = psp.tile([P, TQC], f32, tag="sc")
                            nc.tensor.matmul(
                                ps[:],
                                rk[:, j * P : (j + 1) * P],
                                rq[:, qsl],
                                start=True,
                                stop=True,
                            )
                            et = epl.tile([P, TQC], bf16, tag="e")
                            nc.scalar.activation(et[:], ps[:], EXP, scale=SCALE)
                            if j >= 4 * cc:  # diagonal block: causal mask
                                rr = j - 4 * cc
                                m0 = 384 - 128 * rr
                                nc.vector.tensor_mul(
                                    et[:], et[:], msk_sb[:, m0 : m0 + TQC]
                                )
                            nc.tensor.matmul(
                                psy[:],
                                v_sb[:, j * P : (j + 1) * P],
                                et[:],
                                start=(j == 0),
                                stop=(j == njt - 1),
                                skip_group_check=True,
                            )
                            nc.tensor.matmul(
                                pss[:1, :],
                                oneb_sb[:, :1],
                                et[:],
                                start=(j == 0),
                                stop=(j == njt - 1),
                                skip_group_check=True,
                            )
                        # replicate 1/sums across partitions: ones-matmul + recip
                        ssb = smp.tile([1, TQC], f32r, tag="ssb")
                        nc.vector.tensor_copy(ssb[:1, :], pss[:1, :])
                        psr = pssp.tile([P, TQC], f32, tag="sums")
                        nc.tensor.matmul(
                            psr[:], ones_sb[:1, :], ssb[:1, :], start=True, stop=True
                        )
                        inv = ivp.tile([P, TQC], f32, tag="inv")
                        nc.vector.reciprocal(inv[:], psr[:])
                        nc.vector.tensor_mul(
                            y2_sb[:, hl * T + cc * TQC : hl * T + (cc + 1) * TQC],
                            psy[:],
                            inv[:],
                        )

            # ------------- projection phase (y resident in SBUF) -------------
            with (
                tc.tile_pool(name="wpp", bufs=1) as wpp,
                tc.tile_pool(name="pev", bufs=3) as pvp,
                tc.tile_pool(name="psj", bufs=2, space="PSUM") as psjp,
            ):
                wp_sb = wpp.tile([P, HL * C], f32r, tag="wp")
                for cin in range(HL):
                    nc.sync.dma_start(
                        out=wp_sb[:, cin * C : (cin + 1) * C], in_=wp[:, cin, :]
                    )
                for co in range(NCT):
                    psj = psjp.tile([P, T], f32, tag="pj")
                    for cin in range(HL):
                        lhsT = wp_sb[:, cin * C + co * P : cin * C + (co + 1) * P]
                        for tch in range(NTQ):
                            nc.tensor.matmul(
                                psj[:, tch * TQC : (tch + 1) * TQC],
                                lhsT,
                                y2_sb[:, cin * T + tch * TQC : cin * T + (tch + 1) * TQC],
                                start=(cin == 0),
                                stop=(cin == HL - 1),
                            )
                    pev = pvp.tile([P, T], f32, tag="pev")
                    nc.vector.tensor_copy(pev[:], psj[:])
                    nc.sync.dma_start(out=poutT[co * P : (co + 1) * P, :], in_=pev[:])

            # ---- reduce-scatter within batch pairs, chunked to overlap proj ----
            # chunk g covers poutT rows [512g, 512g+512); rank r of the pair
            # gets rows [512g + 256r, 512g + 256r + 256) -> out rows [256g..)
            if with_collective:
                for g in range(4):
                    nc.gpsimd.collective_compute(
                        "ReduceScatter",
                        mybir.AluOpType.add,
                        replica_groups=[[0, 1], [2, 3], [4, 5], [6, 7]],
                        ins=[poutT[g * 512 : (g + 1) * 512, :]],
                        outs=[rs_out[g * 256 : (g + 1) * 256, :]],
                    )
                    nc.sync.dma_start(
                        out=outp[g * 256 : (g + 1) * 256, :],
                        in_=rs_out[g * 256 : (g + 1) * 256, :],
                    )
            else:  # timeline-sim variant (single-core, no collectives)
                nc.sync.dma_start(out=outp[:, :], in_=poutT[:CLOC, :])

    return nc


def _round_f32r(a):
    u = np.ascontiguousarray(a, np.float32).view(np.uint32)
    u = ((u + np.uint32(0x7FF) + ((u >> np.uint32(12)) & np.uint32(1)))
         & np.uint32(0xFFFFF000))
    return u.view(np.float32)


def _host_tables():
    inv_freq = 1.0 / (ROPE_BASE ** (np.arange(0, D, 2, dtype=np.float64) / D))
    pos = np.arange(T, dtype=np.float64)
    ang = pos[None, :] * inv_freq[:, None]  # [D/2, T]
    cos = np.cos(ang)
    sin = np.sin(ang)
    cost = np.empty((P, T), np.float32)
    sint = np.empty((P, T), np.float32)
    cost[0::2] = cos
    cost[1::2] = cos
    sint[0::2] = -sin
    sint[1::2] = sin
    # shifted causal mask base: diagonal tile r uses cols [384-128r, 896-128r)
    u = np.arange(896) - 384
    mskt = (np.arange(P)[:, None] <= u[None, :]).astype(ml_dtypes.bfloat16)
    swpm = np.zeros((P, P), np.float32)
    for i in range(P):
        swpm[i, i ^ 1] = 1.0
    return cost, sint, mskt, swpm


def _legalize_bir(bir_bytes):
    """Split multi-wait instructions into single-wait NoOps: this container's
    walrus codegen rejects >1 sync wait on f32/f32r matmuls and drains."""
    import json as _json

    bir = _json.loads(bir_bytes)
    n = 0
    for f in bir.get("functions", []):
        for b in f.get("blocks", []):
            new = []
            for inst in b["instructions"]:
                si = inst.get("sync_info") or {}
                waits = si.get("on_wait") or []
                if len(waits) > 1 and inst.get("engine"):
                    for w in waits[:-1]:
                        n += 1
                        new.append(
                            {
                                "name": f"{inst['name']}.lw{n}",
                                "opcode": "NoOp",
                                "engine": inst["engine"],
                                "ins": [],
                                "outs": [],
                                "sync_info": {"on_update": [], "on_wait": [w]},
                            }
                        )
                    si["on_wait"] = [waits[-1]]
                    inst["sync_info"] = si
                new.append(inst)
            b["instructions"] = new
    return _json.dumps(bir).encode()


def _install_compile_patch():
    import concourse.bass2jax as _b2j
    import concourse.bass_utils as _bu

    if getattr(_bu.compile_bir_kernel, "_legalized", False):
        return
    _orig = _bu.compile_bir_kernel

    def _patched(bir_json, tmpdir, neff_name="file.neff"):
        return _orig(_legalize_bir(bir_json), tmpdir, neff_name=neff_name)

    _patched._legalized = True
    _bu.compile_bir_kernel = _patched
    _b2j.compile_bir_kernel = _patched


_install_compile_patch()

_NC_CACHE = {}
_PROFILE = {"on": False, "exec_time_ns": None, "trace_dir": None, "times_ms": None}


def _run_timed(nc, in_maps, n_cores=8, iters=12):
    """Mirror bass2jax.run_bass_via_pjrt's multi-core path, but keep inputs
    on device and time repeated dispatches (no NTFF hook in this container)."""
    import time

    import jax
    from jax.experimental.shard_map import shard_map
    from jax.sharding import Mesh, NamedSharding, PartitionSpec

    from concourse import mybir as _mb
    from concourse.bass2jax import (
        _bass_exec_p,
        install_neuronx_cc_hook,
        partition_id_tensor,
    )

    install_neuronx_cc_hook()
    partition_name = nc.partition_id_tensor.name if nc.partition_id_tensor else None
    in_names, out_names, out_avals, zero_outs = [], [], [], []
    for alloc in nc.m.functions[0].allocations:
        if not isinstance(alloc, _mb.MemoryLocationSet):
            continue
        name = alloc.memorylocations[0].name
        if alloc.kind == "ExternalInput":
            if name != partition_name:
                in_names.append(name)
        elif alloc.kind == "ExternalOutput":
            out_names.append(name)
            shape = tuple(alloc.tensor_shape)
            dtype = _mb.dt.np(alloc.dtype)
            out_avals.append(jax.core.ShapedArray(shape, dtype))
            zero_outs.append(np.zeros(shape, dtype))
    n_params = len(in_names)
    all_in_names = list(in_names) + list(out_names)
    if partition_name is not None:
        all_in_names.append(partition_name)

    def _body(*args):
        operands = list(args)
        if partition_name is not None:
            operands.append(partition_id_tensor())
        outs = _bass_exec_p.bind(
            *operands,
            out_avals=tuple(out_avals),
            in_names=tuple(all_in_names),
            out_names=tuple(out_names),
            lowering_input_output_aliases=(),
            sim_require_finite=True,
            sim_require_nnan=True,
            nc=nc,
        )
        return tuple(outs)

    devices = jax.devices()[:n_cores]
    mesh = Mesh(np.asarray(devices), ("core",))
    spec = NamedSharding(mesh, PartitionSpec("core"))
    n_outs = len(out_avals)
    sharded = jax.jit(
        shard_map(
            _body,
            mesh=mesh,
            in_specs=(PartitionSpec("core"),) * (n_params + n_outs),
            out_specs=(PartitionSpec("core"),) * n_outs,
            check_rep=False,
        ),
        keep_unused=True,
    )
    concat_in = [
        jax.device_put(
            np.concatenate([np.asarray(in_maps[c][name]) for c in range(n_cores)], 0),
            spec,
        )
        for name in in_names
    ]
    concat_zeros = [
        jax.device_put(np.zeros((n_cores * z.shape[0], *z.shape[1:]), z.dtype), spec)
        for z in zero_outs
    ]
    out_arrs = sharded(*concat_in, *concat_zeros)  # warmup/compile
    jax.block_until_ready(out_arrs)
    times = []
    for _ in range(iters):
        t0 = time.perf_counter()
        r = sharded(*concat_in, *concat_zeros)
        jax.block_until_ready(r)
        times.append(time.perf_counter() - t0)
    _PROFILE["exec_time_ns"] = int(min(times) * 1e9)
    _PROFILE["times_ms"] = [t * 1e3 for t in times]
    results = [
        {
            name: np.asarray(out_arrs[i]).reshape(n_cores, *out_avals[i].shape)[c]
            for i, name in enumerate(out_names)
        }
        for c in range(n_cores)
    ]

    class _R:
        pass

    rr = _R()
    rr.results = results
    return rr


def kernel(x, Wqkv, Wproj):
    if "nc" not in _NC_CACHE:
        _NC_CACHE["nc"] = build_nc()
    nc = _NC_CACHE["nc"]

    x = np.asarray(x, np.float32)
    Wqkv = np.asarray(Wqkv, np.float32)
    Wproj = _round_f32r(np.asarray(Wproj, np.float32))
    cost, sint, mskt, swpm = _host_tables()
    cost = _round_f32r(cost)
    sint = _round_f32r(sint)
    idnm = np.eye(P, dtype=ml_dtypes.bfloat16)
    onem_np = np.ones((P, P), np.float32)
    onbm_np = np.ones((P, P), ml_dtypes.bfloat16)

    Wq, Wk, Wv = Wqkv[:, 0:C], Wqkv[:, C : 2 * C], Wqkv[:, 2 * C : 3 * C]

    def wtile(Wm, hf):  # [C, 1024] -> [HL, P, NCT, P] bf16
        Wl = Wm[:, hf * CLOC : (hf + 1) * CLOC]
        return np.ascontiguousarray(
            Wl.reshape(NCT, P, HL, P).transpose(2, 1, 0, 3).astype(ml_dtypes.bfloat16)
        )

    in_maps = []
    for c in range(8):
        b, hf = c // 2, c % 2
        xTc = np.ascontiguousarray(
            x[b].T.reshape(NCT, P, T).transpose(1, 0, 2).astype(ml_dtypes.bfloat16)
        )  # [P, NCT, T]
        wpc = np.ascontiguousarray(
            Wproj[hf * CLOC : (hf + 1) * CLOC, :].reshape(HL, P, C).transpose(1, 0, 2)
        )  # [P, HL, C]
        in_maps.append(
            {
                "xT": xTc,
                "wq": wtile(Wq, hf),
                "wk": wtile(Wk, hf),
                "wv": wtile(Wv, hf),
                "wp": wpc,
                "cost": cost,
                "sint": sint,
                "mskt": mskt,
                "swpm": swpm,
                "idnm": idnm,
                "onem": onem_np,
                "onbm": onbm_np,
            }
        )

    if _PROFILE.get("on"):
        res = _run_timed(nc, in_maps)
    else:
        res = run_bass_kernel_spmd(nc, in_maps, core_ids=list(range(8)))
    out = np.empty((B, T, C), np.float32)
    for c in range(8):
        b, hf = c // 2, c % 2
        r = res.results[c]["out"]  # [1024, T]: 4 chunks of 256 cout rows
        for g in range(4):
            cout0 = g * 512 + hf * 256
            out[b, :, cout0 : cout0 + 256] = r[g * 256 : (g + 1) * 256].T
    return out


if __name__ == "__main__":
    nc = build_nc()
    print("graph built ok:", len(nc.m.functions[0].allocations), "allocations")


# revision 22
# speedup vs baseline: 51.8691x; 51.8691x over previous
"""Distributed causal self-attention kernel for 8 TRN2 NeuronCores.

Sharding: core c handles batch b=c//2 and head-half hf=c%2 (8 of 16 heads).
Per core: qkv projection (bf16 matmuls into f32 PSUM), RoPE in f32r (PE
swap-matmul + DVE), causal flash attention in [tk, tq] layout (scores f32r;
exp -> bf16 weights; softmax sums via a ones-matmul; normalization deferred
to the y eviction), output projection in f32r producing the TRANSPOSED
partial output [C, T], then 4 chunked 2-way ReduceScatters within each batch
pair overlapping the projection. y stays resident in SBUF (no HBM round
trip). Multi-wait instructions are legalized into single-wait NoOps because
this container's walrus rejects them.
"""

import sys

sys.path.insert(0, "/opt/trn_rl_repo")

import ml_dtypes
import numpy as np

import concourse.bass as bass
import concourse.mybir as mybir
from concourse.bass_utils import run_bass_kernel_spmd
from concourse.tile import TileContext

# Problem constants (hardcoded; kernel.py must be self-contained)
B, T, C = 4, 2048, 2048
H, D = 16, 128
HL = 8  # local heads per core
CLOC = HL * D  # 1024 local y features
ROPE_BASE = 500000
P = 128
NCT = C // P  # 16 contraction tiles
TQC = 512  # tq chunk
NTQ = T // TQC  # 4
NTK = T // P  # 16 tk tiles
SCALE = 1.0 / float(np.sqrt(D))

f32 = mybir.dt.float32
f32r = mybir.dt.float32r
bf16 = mybir.dt.bfloat16
EXP = mybir.ActivationFunctionType.Exp


def build_nc(with_collective=True):
    nc = bass.Bass(target_bir_lowering=False, num_devices=8)

    # per-core parameters (host pre-arranged layouts, all contiguous DMAs)
    xT = nc.declare_dram_parameter("xT", [P, NCT, T], bf16, isOutput=False)
    wq = nc.declare_dram_parameter("wq", [HL, P, NCT, P], bf16, isOutput=False)
    wk = nc.declare_dram_parameter("wk", [HL, P, NCT, P], bf16, isOutput=False)
    wv = nc.declare_dram_parameter("wv", [HL, P, NCT, P], bf16, isOutput=False)
    wp = nc.declare_dram_parameter("wp", [P, HL, C], f32r, isOutput=False)
    cost = nc.declare_dram_parameter("cost", [P, T], f32r, isOutput=False)
    sint = nc.declare_dram_parameter("sint", [P, T], f32r, isOutput=False)
    mskt = nc.declare_dram_parameter("mskt", [P, 896], bf16, isOutput=False)
    swpm = nc.declare_dram_parameter("swpm", [P, P], f32r, isOutput=False)
    idnm = nc.declare_dram_parameter("idnm", [P, P], bf16, isOutput=False)
    onem = nc.declare_dram_parameter("onem", [P, P], f32r, isOutput=False)
    onbm = nc.declare_dram_parameter("onbm", [P, P], bf16, isOutput=False)
    outp = nc.declare_dram_parameter("out", [CLOC, T], f32, isOutput=True)

    poutT = nc.dram_tensor("poutT", [C, T], f32)
    rs_out = nc.dram_tensor("rs_out", [CLOC, T], f32)

    with TileContext(nc) as tc:
        with (
            tc.tile_pool(name="const", bufs=1) as cp,
            tc.tile_pool(name="ybuf", bufs=1) as yp,
            tc.tile_pool(name="xt", bufs=1) as xtp,
        ):
            cos_sb = cp.tile([P, T], f32r, tag="cos")
            sin_sb = cp.tile([P, T], f32r, tag="sin")
            msk_sb = cp.tile([P, 896], bf16, tag="msk")
            swp_sb = cp.tile([P, P], f32r, tag="swp")
            idn_sb = cp.tile([P, P], bf16, tag="idn")
            ones_sb = cp.tile([P, P], f32r, tag="ones1")
            oneb_sb = cp.tile([P, P], bf16, tag="ones2")
            # y (normalized, f32r) stays resident across attention + proj
            y2_sb = yp.tile([P, HL * T], f32r, tag="y2")

            # ---------------- attention phase ----------------
            with (
                tc.tile_pool(name="wqk", bufs=4) as wqkp,
                tc.tile_pool(name="qk", bufs=3) as qkp,
                tc.tile_pool(name="rtmp", bufs=3) as rtp,
                tc.tile_pool(name="e", bufs=4) as epl,
                tc.tile_pool(name="sum", bufs=2) as smp,
                tc.tile_pool(name="inv", bufs=3) as ivp,
                tc.tile_pool(name="ps", bufs=2, space="PSUM") as psp,
                tc.tile_pool(name="psa", bufs=2, space="PSUM") as psap,
                tc.tile_pool(name="psy", bufs=1, space="PSUM") as psyp,
                tc.tile_pool(name="pss", bufs=1, space="PSUM") as pssp,
            ):
                # prefetch head-0's wv before the big xt load so the PE
                # can start as soon as the first xt chunks land
                wv_first = wqkp.tile([P, NCT * P], bf16, tag="wqk")
                nc.sync.dma_start(
                    out=wv_first[:], in_=wv[0].rearrange("p n d -> p (n d)")
                )
                xt_sb = xtp.tile([P, NCT * T], bf16, tag="xt")
                for ci in range(0, NCT, 2):
                    nc.sync.dma_start(
                        out=xt_sb[:, ci * T : (ci + 2) * T],
                        in_=xT[:, ci : ci + 2, :].rearrange("p n t -> p (n t)"),
                    )
                # consts are needed later than xt/weights: issue after
                nc.sync.dma_start(out=idn_sb[:], in_=idnm[:, :])
                nc.sync.dma_start(out=cos_sb[:], in_=cost[:, :])
                nc.sync.dma_start(out=sin_sb[:], in_=sint[:, :])
                nc.sync.dma_start(out=msk_sb[:], in_=mskt[:, :])
                nc.sync.dma_start(out=swp_sb[:], in_=swpm[:, :])
                nc.sync.dma_start(out=ones_sb[:], in_=onem[:, :])
                nc.sync.dma_start(out=oneb_sb[:], in_=onbm[:, :])

                def qk_acc(w_sb, dst):
                    # dst[:, t] = sum_c w[c, d].T @ xT[c, t], two 2-bank halves
                    for hh in range(2):
                        ps = psap.tile([P, 2 * TQC], f32, tag="qkacc")
                        for ci in range(NCT):
                            lhsT = w_sb[:, ci * P : (ci + 1) * P]
                            for tt in range(2):
                                t0 = (hh * 2 + tt) * TQC
                                nc.tensor.matmul(
                                    ps[:, tt * TQC : (tt + 1) * TQC],
                                    lhsT,
                                    xt_sb[:, ci * T + t0 : ci * T + t0 + TQC],
                                    start=(ci == 0),
                                    stop=(ci == NCT - 1),
                                )
                        nc.vector.tensor_copy(
                            dst[:, hh * 2 * TQC : (hh + 1) * 2 * TQC], ps[:]
                        )

                def rope(src):
                    # in-place: src = src*cos + swap(src)*sin (sin table signed;
                    # the 1/sqrt(D) scale is folded into the exp activation)
                    for cc in range(NTQ):
                        sl = slice(cc * TQC, (cc + 1) * TQC)
                        ps = psp.tile([P, TQC], f32, tag="sc")
                        nc.tensor.matmul(
                            ps[:], swp_sb[:], src[:, sl], start=True, stop=True
                        )
                        tmp = rtp.tile([P, TQC], f32r, tag="rtmp")
                        nc.vector.tensor_mul(tmp[:], ps[:], sin_sb[:, sl])
                        nc.vector.tensor_mul(src[:, sl], src[:, sl], cos_sb[:, sl])
                        nc.vector.tensor_add(src[:, sl], src[:, sl], tmp[:])

                for hl in range(HL):
                    # v via the same machinery (transposed layout), then
                    # PE-transpose per 128-tile into natural [tk, d], all bf16
                    if hl == 0:
                        wv_sb = wv_first
                    else:
                        wv_sb = wqkp.tile([P, NCT * P], bf16, tag="wqk")
                        nc.sync.dma_start(
                            out=wv_sb[:], in_=wv[hl].rearrange("p n d -> p (n d)")
                        )
                    vT = qkp.tile([P, T], bf16, tag="qk")
                    qk_acc(wv_sb, vT)
                    v_sb = qkp.tile([P, T], bf16, tag="qk")
                    for j in range(NTK):
                        pst = psp.tile([P, P], bf16, tag="sc")
                        nc.tensor.transpose(
                            pst[:], vT[:, j * P : (j + 1) * P], idn_sb[:]
                        )
                        nc.vector.tensor_copy(v_sb[:, j * P : (j + 1) * P], pst[:])

                    wk_sb = wqkp.tile([P, NCT * P], bf16, tag="wqk")
                    nc.sync.dma_start(
                        out=wk_sb[:], in_=wk[hl].rearrange("p n d -> p (n d)")
                    )
                    rk = qkp.tile([P, T], f32r, tag="qk")
                    qk_acc(wk_sb, rk)
                    rope(rk)

                    wq_sb = wqkp.tile([P, NCT * P], bf16, tag="wqk")
                    nc.sync.dma_start(
                        out=wq_sb[:], in_=wq[hl].rearrange("p n d -> p (n d)")
                    )
                    rq = qkp.tile([P, T], f32r, tag="qk")
                    qk_acc(wq_sb, rq)
                    rope(rq)

                    for cc in range(NTQ):
                        qsl = slice(cc * TQC, (cc + 1) * TQC)
                        njt = 4 * cc + 4  # causal tk tiles
                        psy = psyp.tile([P, TQC], f32, tag="yac")
                        pss = pssp.tile([1, TQC], f32, tag="sums")
                        for j in range(njt):
                            ps = psp.tile([P, TQC], f32, tag="sc")
                            nc.tensor.matmul(
                                ps[:],
                                rk[:, j * P : (j + 1) * P],
                                rq[:, qsl],
                                start=True,
                                stop=True,
                            )
                            et = epl.tile([P, TQC], bf16, tag="e")
                            nc.scalar.activation(et[:], ps[:], EXP, scale=SCALE)
                            if j >= 4 * cc:  # diagonal block: causal mask
                                rr = j - 4 * cc
                                m0 = 384 - 128 * rr
                                nc.vector.tensor_mul(
                                    et[:], et[:], msk_sb[:, m0 : m0 + TQC]
                                )
                            nc.tensor.matmul(
                                psy[:],
                                v_sb[:, j * P : (j + 1) * P],
                                et[:],
                                start=(j == 0),
                                stop=(j == njt - 1),
                                skip_group_check=True,
                            )
                            nc.tensor.matmul(
                                pss[:1, :],
                                oneb_sb[:, :1],
                                et[:],
                                start=(j == 0),
                                stop=(j == njt - 1),
                                skip_group_check=True,
                            )
                        # replicate 1/sums across partitions: ones-matmul + recip
                        ssb = smp.tile([1, TQC], f32r, tag="ssb")
                        nc.vector.tensor_copy(ssb[:1, :], pss[:1, :])
                        psr = pssp.tile([P, TQC], f32, tag="sums")
                        nc.tensor.matmul(
                            psr[:], ones_sb[:1, :], ssb[:1, :], start=True, stop=True
                        )
                        inv = ivp.tile([P, TQC], f32, tag="inv")
                        nc.vector.reciprocal(inv[:], psr[:])
                        nc.vector.tensor_mul(
                            y2_sb[:, hl * T + cc * TQC : hl * T + (cc + 1) * TQC],
                            psy[:],
                            inv[:],
                        )

                # proj weights reuse the xt slot as soon as head 7 frees it
                wp_sb = xtp.tile([P, HL * C], f32r, tag="xt")
                for cin in range(HL):
                    nc.sync.dma_start(
                        out=wp_sb[:, cin * C : (cin + 1) * C], in_=wp[:, cin, :]
                    )

            # ------------- projection phase (y resident in SBUF) -------------
            with (
                tc.tile_pool(name="pev", bufs=3) as pvp,
                tc.tile_pool(name="psj", bufs=2, space="PSUM") as psjp,
            ):
                for co in range(NCT):
                    psj = psjp.tile([P, T], f32, tag="pj")
                    for cin in range(HL):
                        lhsT = wp_sb[:, cin * C + co * P : cin * C + (co + 1) * P]
                        for tch in range(NTQ):
                            nc.tensor.matmul(
                                psj[:, tch * TQC : (tch + 1) * TQC],
                                lhsT,
                                y2_sb[:, cin * T + tch * TQC : cin * T + (tch + 1) * TQC],
                                start=(cin == 0),
                                stop=(cin == HL - 1),
                            )
                    pev = pvp.tile([P, T], f32, tag="pev")
                    nc.vector.tensor_copy(pev[:], psj[:])
                    nc.sync.dma_start(out=poutT[co * P : (co + 1) * P, :], in_=pev[:])

            # ---- reduce-scatter within batch pairs, chunked to overlap proj ----
            # chunk g covers poutT rows [512g, 512g+512); rank r of the pair
            # gets rows [512g + 256r, 512g + 256r + 256) -> out rows [256g..)
            if with_collective:
                for g in range(4):
                    nc.gpsimd.collective_compute(
                        "ReduceScatter",
                        mybir.AluOpType.add,
                        replica_groups=[[0, 1], [2, 3], [4, 5], [6, 7]],
                        ins=[poutT[g * 512 : (g + 1) * 512, :]],
                        outs=[rs_out[g * 256 : (g + 1) * 256, :]],
                    )
                    nc.sync.dma_start(
                        out=outp[g * 256 : (g + 1) * 256, :],
                        in_=rs_out[g * 256 : (g + 1) * 256, :],
                    )
            else:  # timeline-sim variant (single-core, no collectives)
                nc.sync.dma_start(out=outp[:, :], in_=poutT[:CLOC, :])

    return nc


def _round_f32r(a):
    u = np.ascontiguousarray(a, np.float32).view(np.uint32)
    u = ((u + np.uint32(0x7FF) + ((u >> np.uint32(12)) & np.uint32(1)))
         & np.uint32(0xFFFFF000))
    return u.view(np.float32)


def _host_tables():
    inv_freq = 1.0 / (ROPE_BASE ** (np.arange(0, D, 2, dtype=np.float64) / D))
    pos = np.arange(T, dtype=np.float64)
    ang = pos[None, :] * inv_freq[:, None]  # [D/2, T]
    cos = np.cos(ang)
    sin = np.sin(ang)
    cost = np.empty((P, T), np.float32)
    sint = np.empty((P, T), np.float32)
    cost[0::2] = cos
    cost[1::2] = cos
    sint[0::2] = -sin
    sint[1::2] = sin
    # shifted causal mask base: diagonal tile r uses cols [384-128r, 896-128r)
    u = np.arange(896) - 384
    mskt = (np.arange(P)[:, None] <= u[None, :]).astype(ml_dtypes.bfloat16)
    swpm = np.zeros((P, P), np.float32)
    for i in range(P):
        swpm[i, i ^ 1] = 1.0
    return cost, sint, mskt, swpm


def _legalize_bir(bir_bytes):
    """Split multi-wait instructions into single-wait NoOps: this container's
    walrus codegen rejects >1 sync wait on f32/f32r matmuls and drains."""
    import json as _json

    bir = _json.loads(bir_bytes)
    n = 0
    for f in bir.get("functions", []):
        for b in f.get("blocks", []):
            new = []
            for inst in b["instructions"]:
                si = inst.get("sync_info") or {}
                waits = si.get("on_wait") or []
                if len(waits) > 1 and inst.get("engine"):
                    for w in waits[:-1]:
                        n += 1
                        new.append(
                            {
                                "name": f"{inst['name']}.lw{n}",
                                "opcode": "NoOp",
                                "engine": inst["engine"],
                                "ins": [],
                                "outs": [],
                                "sync_info": {"on_update": [], "on_wait": [w]},
                            }
                        )
                    si["on_wait"] = [waits[-1]]
                    inst["sync_info"] = si
                new.append(inst)
            b["instructions"] = new
    return _json.dumps(bir).encode()


def _install_compile_patch():
    import concourse.bass2jax as _b2j
    import concourse.bass_utils as _bu

    if getattr(_bu.compile_bir_kernel, "_legalized", False):
        return
    _orig = _bu.compile_bir_kernel

    def _patched(bir_json, tmpdir, neff_name="file.neff"):
        return _orig(_legalize_bir(bir_json), tmpdir, neff_name=neff_name)

    _patched._legalized = True
    _bu.compile_bir_kernel = _patched
    _b2j.compile_bir_kernel = _patched


_install_compile_patch()

_NC_CACHE = {}
_PROFILE = {"on": False, "exec_time_ns": None, "trace_dir": None, "times_ms": None}


def _run_timed(nc, in_maps, n_cores=8, iters=12):
    """Mirror bass2jax.run_bass_via_pjrt's multi-core path, but keep inputs
    on device and time repeated dispatches (no NTFF hook in this container)."""
    import time

    import jax
    from jax.experimental.shard_map import shard_map
    from jax.sharding import Mesh, NamedSharding, PartitionSpec

    from concourse import mybir as _mb
    from concourse.bass2jax import (
        _bass_exec_p,
        install_neuronx_cc_hook,
        partition_id_tensor,
    )

    install_neuronx_cc_hook()
    partition_name = nc.partition_id_tensor.name if nc.partition_id_tensor else None
    in_names, out_names, out_avals, zero_outs = [], [], [], []
    for alloc in nc.m.functions[0].allocations:
        if not isinstance(alloc, _mb.MemoryLocationSet):
            continue
        name = alloc.memorylocations[0].name
        if alloc.kind == "ExternalInput":
            if name != partition_name:
                in_names.append(name)
        elif alloc.kind == "ExternalOutput":
            out_names.append(name)
            shape = tuple(alloc.tensor_shape)
            dtype = _mb.dt.np(alloc.dtype)
            out_avals.append(jax.core.ShapedArray(shape, dtype))
            zero_outs.append(np.zeros(shape, dtype))
    n_params = len(in_names)
    all_in_names = list(in_names) + list(out_names)
    if partition_name is not None:
        all_in_names.append(partition_name)

    def _body(*args):
        operands = list(args)
        if partition_name is not None:
            operands.append(partition_id_tensor())
        outs = _bass_exec_p.bind(
            *operands,
            out_avals=tuple(out_avals),
            in_names=tuple(all_in_names),
            out_names=tuple(out_names),
            lowering_input_output_aliases=(),
            sim_require_finite=True,
            sim_require_nnan=True,
            nc=nc,
        )
        return tuple(outs)

    devices = jax.devices()[:n_cores]
    mesh = Mesh(np.asarray(devices), ("core",))
    spec = NamedSharding(mesh, PartitionSpec("core"))
    n_outs = len(out_avals)
    sharded = jax.jit(
        shard_map(
            _body,
            mesh=mesh,
            in_specs=(PartitionSpec("core"),) * (n_params + n_outs),
            out_specs=(PartitionSpec("core"),) * n_outs,
            check_rep=False,
        ),
        keep_unused=True,
    )
    concat_in = [
        jax.device_put(
            np.concatenate([np.asarray(in_maps[c][name]) for c in range(n_cores)], 0),
            spec,
        )
        for name in in_names
    ]
    concat_zeros = [
        jax.device_put(np.zeros((n_cores * z.shape[0], *z.shape[1:]), z.dtype), spec)
        for z in zero_outs
    ]
    out_arrs = sharded(*concat_in, *concat_zeros)  # warmup/compile
    jax.block_until_ready(out_arrs)
    times = []
    for _ in range(iters):
        t0 = time.perf_counter()
        r = sharded(*concat_in, *concat_zeros)
        jax.block_until_ready(r)
        times.append(time.perf_counter() - t0)
    _PROFILE["exec_time_ns"] = int(min(times) * 1e9)
    _PROFILE["times_ms"] = [t * 1e3 for t in times]
    results = [
        {
            name: np.asarray(out_arrs[i]).reshape(n_cores, *out_avals[i].shape)[c]
            for i, name in enumerate(out_names)
        }
        for c in range(n_cores)
    ]

    class _R:
        pass

    rr = _R()
    rr.results = results
    return rr


def kernel(x, Wqkv, Wproj):
    if "nc" not in _NC_CACHE:
        _NC_CACHE["nc"] = build_nc()
    nc = _NC_CACHE["nc"]

    x = np.asarray(x, np.float32)
    Wqkv = np.asarray(Wqkv, np.float32)
    Wproj = _round_f32r(np.asarray(Wproj, np.float32))
    cost, sint, mskt, swpm = _host_tables()
    cost = _round_f32r(cost)
    sint = _round_f32r(sint)
    idnm = np.eye(P, dtype=ml_dtypes.bfloat16)
    onem_np = np.ones((P, P), np.float32)
    onbm_np = np.ones((P, P), ml_dtypes.bfloat16)

    Wq, Wk, Wv = Wqkv[:, 0:C], Wqkv[:, C : 2 * C], Wqkv[:, 2 * C : 3 * C]

    def wtile(Wm, hf):  # [C, 1024] -> [HL, P, NCT, P] bf16
        Wl = Wm[:, hf * CLOC : (hf + 1) * CLOC]
        return np.ascontiguousarray(
            Wl.reshape(NCT, P, HL, P).transpose(2, 1, 0, 3).astype(ml_dtypes.bfloat16)
        )

    in_maps = []
    for c in range(8):
        b, hf = c // 2, c % 2
        xTc = np.ascontiguousarray(
            x[b].T.reshape(NCT, P, T).transpose(1, 0, 2).astype(ml_dtypes.bfloat16)
        )  # [P, NCT, T]
        wpc = np.ascontiguousarray(
            Wproj[hf * CLOC : (hf + 1) * CLOC, :].reshape(HL, P, C).transpose(1, 0, 2)
        )  # [P, HL, C]
        in_maps.append(
            {
                "xT": xTc,
                "wq": wtile(Wq, hf),
                "wk": wtile(Wk, hf),
                "wv": wtile(Wv, hf),
                "wp": wpc,
                "cost": cost,
                "sint": sint,
                "mskt": mskt,
                "swpm": swpm,
                "idnm": idnm,
                "onem": onem_np,
                "onbm": onbm_np,
            }
        )

    if _PROFILE.get("on"):
        res = _run_timed(nc, in_maps)
    else:
        res = run_bass_kernel_spmd(nc, in_maps, core_ids=list(range(8)))
    out = np.empty((B, T, C), np.float32)
    for c in range(8):
        b, hf = c // 2, c % 2
        r = res.results[c]["out"]  # [1024, T]: 4 chunks of 256 cout rows
        for g in range(4):
            cout0 = g * 512 + hf * 256
            out[b, :, cout0 : cout0 + 256] = r[g * 256 : (g + 1) * 256].T
    return out


if __name__ == "__main__":
    nc = build_nc()
    print("graph built ok:", len(nc.m.functions[0].allocations), "allocations")
